# revision 1
# baseline (speedup 1.0000x reference)
"""Distributed GQA attention kernel for one TRN2 chip (8 NeuronCores).

nn_Attention: B=2, S=2048, D=2048, H=32 q-heads, KV=8 kv-heads, HD=64,
RoPE (interleaved pairs), causal softmax, GQA repeat 4, output proj.

Sharding (tensor-parallel over heads): core c owns q-heads 4c..4c+3 and
kv-head c. x and freq tables replicated. Instead of an AllReduce after wo,
each core's per-head attention output is exchanged with an AllToAll (bf16,
1/16 the AllReduce bytes) so that core c ends up with the full attention
activation for tokens [256c:256c+256) of each batch, then computes the wo
projection for just those tokens. Host concatenates the 8 token slices.

Per-core schedule (engines execute in emission order, so cross-phase
overlap comes from interleaved emission):
  1. QKV+RoPE for batch-0 tokens: x cast-DMA'd f32->bf16 by SWDGE,
     PE-transposed to d-major (evictions on ACT here, where ACT is idle);
     fused QKV matmuls with host-transposed / RoPE-deinterleaved weights;
     RoPE on DVE straight from PSUM; V transposed token-major with a ones
     column appended (softmax-denominator trick).
  2. Batch-0 attention, emission-interleaved chunk-by-chunk with batch-1
     QKV+RoPE so exp (ACT-bound) overlaps projection work (PE/DVE-bound):
     scores^T with K stationary, exp on ACT from PSUM with no max
     subtraction (|scores| < 6 at this problem's scale), causal zeroing of
     diagonal blocks via gpsimd affine_select post-exp, PV with expS^T
     stationary and V_aug moving (65th column accumulates the softmax
     denominator per q-partition), reciprocal + per-partition scale,
     PE-transpose to e-major, DMA into A2A chunks.
  3. AllToAll(batch 0); batch-1 attention (wo tiles DMA-prefetched
     meanwhile; the collective flies under it).
  4. AllToAll(batch 1); wo matmul per batch with the received activation
     stationary and host-transposed wo moving -> token-major output.
"""
from contextlib import ExitStack

import numpy as np

import concourse.bass as bass
import concourse.mybir as mybir
import concourse.tile as tile
from concourse import bacc
from concourse.bass_utils import run_bass_kernel_spmd
from concourse.masks import make_identity

F32 = mybir.dt.float32
BF16 = mybir.dt.bfloat16
AF = mybir.ActivationFunctionType

NC_CORES = 8
B = 2
S = 2048
D = 2048
H = 32
KV = 8
HD = 64
HPC = H // NC_CORES      # 4 q heads per core
EQ = HPC * HD            # 256
T = B * S
TB = 512                 # phase-1 token block
NTB = T // TB
KTILES = S // 128
DT = D // 128
TSLICE = T // NC_CORES
BSL = TSLICE // B        # per-batch token slice each core outputs
QSPAN = 512


def build(reps: int = 1, timeline: bool = False):
    nc = bacc.Bacc("TRN2", target_bir_lowering=False, debug=False,
                   num_devices=NC_CORES)

    x = nc.dram_tensor("x", [T, D], F32, kind="ExternalInput")
    cos4 = nc.dram_tensor("cos4", [128, S], F32, kind="ExternalInput")
    sin4 = nc.dram_tensor("sin4", [128, S], F32, kind="ExternalInput")
    wqTA = nc.dram_tensor("wqTA", [D, 128], F32, kind="ExternalInput")
    wqTB = nc.dram_tensor("wqTB", [D, 128], F32, kind="ExternalInput")
    wkvT = nc.dram_tensor("wkvT", [D, 128], F32, kind="ExternalInput")
    woT = nc.dram_tensor("woT", [D, D], F32, kind="ExternalInput")
    out = nc.dram_tensor("out", [TSLICE, D], F32, kind="ExternalOutput")

    a2a_in = [nc.dram_tensor(f"a2a_in{b}", [NC_CORES, EQ, BSL], BF16)
              for b in range(B)]
    a2a_out = [nc.dram_tensor(f"a2a_out{b}", [NC_CORES, EQ, BSL], BF16)
               for b in range(B)]
    rg = [list(range(NC_CORES))]

    with tile.TileContext(nc) as tc, ExitStack() as es:
        const = es.enter_context(tc.tile_pool(name="const", bufs=1))
        ident = const.tile([128, 128], BF16, tag="ident")
        make_identity(nc, ident[:])
        ones_c = const.tile([1, 64], BF16, tag="ones_c")
        nc.gpsimd.memset(ones_c[:], 1.0)

        qt_pool = es.enter_context(tc.tile_pool(name="qt", bufs=1))
        QTb = [[qt_pool.tile([128, S], BF16, tag=f"QT{b}{g}", name=f"QT{b}{g}")
                for g in range(2)] for b in range(B)]
        KTb = [qt_pool.tile([128, S], BF16, tag=f"KT{b}", name=f"KT{b}")
               for b in range(B)]

        vpool = es.enter_context(tc.tile_pool(name="vaug", bufs=B * KTILES))
        V_aug = []
        for i in range(B * KTILES):
            v = vpool.tile([128, 65], BF16, tag="vaug")
            nc.gpsimd.memset(v[:, 64:65], 1.0)
            V_aug.append(v)

        for _rep in range(reps):
          with tc.tile_pool(name="att", bufs=2) as att, \
               tc.tile_pool(name="expp", bufs=2) as expp, \
               tc.tile_pool(name="psATs", bufs=2, space="PSUM") as psATp:

            # ---------------- emit helpers ----------------
            def p1_chunks(tb, p1sb, xbfp, xtp, p1ps, cos_sb, sin_sb,
                          wq_sb_A, wq_sb_B, wkv_sb):
                """List of closures; calling all in order emits phase 1
                (load, transpose, QKV, RoPE, V) for token block tb."""
                t0 = tb * TB
                state = {}

                def do_transpose(dt, evict_act):
                    psT = psATp.tile([128, TB], BF16, tag="psT",
                                     name=f"psT{tb}_{dt}")
                    for i in range(4):
                        nc.tensor.transpose(
                            psT[:, 128 * i: 128 * (i + 1)],
                            state["xbf"][i][:, 128 * dt: 128 * (dt + 1)],
                            ident[:])
                    xt_ = xtp.tile([128, TB], BF16, tag="xT",
                                   name=f"xT{tb}_{dt}")
                    if evict_act:
                        nc.scalar.copy(xt_[:], psT[:])
                    else:
                        nc.vector.tensor_copy(xt_[:], psT[:])
                    return xt_

                def load_dma():
                    xbf = []
                    for i in range(4):
                        xt_ = xbfp.tile([128, D], BF16, tag="xbf",
                                        name=f"xbf{tb}_{i}")
                        nc.gpsimd.dma_start(
                            xt_[:], x[t0 + 128 * i: t0 + 128 * (i + 1), :])
                        xbf.append(xt_)
                    state["xbf"] = xbf

                def load_alloc():
                    state["psQA"] = p1ps.tile([128, TB], F32, tag="psQA",
                                              name=f"psQA{tb}")
                    state["psQB"] = p1ps.tile([128, TB], F32, tag="psQB",
                                              name=f"psQB{tb}")
                    state["psKV"] = p1ps.tile([128, TB], F32, tag="psKV",
                                              name=f"psKV{tb}")
                    state["xT"] = do_transpose(0, tb < 4)

                def qkv(dt):
                    def f():
                        xt_ = state["xT"]
                        if dt + 1 < DT:
                            state["xT"] = do_transpose(dt + 1, tb < 4)
                        st = dict(start=(dt == 0), stop=(dt == DT - 1))
                        nc.tensor.matmul(state["psQA"][:], wq_sb_A[:, dt, :],
                                         xt_[:], **st)
                        nc.tensor.matmul(state["psQB"][:], wq_sb_B[:, dt, :],
                                         xt_[:], **st)
                        nc.tensor.matmul(state["psKV"][:], wkv_sb[:, dt, :],
                                         xt_[:], **st)
                    return f

                def rope():
                    psQA, psQB, psKV = state["psQA"], state["psQB"], state["psKV"]
                    s0 = t0 % S
                    cs = cos_sb[:, s0:s0 + TB]
                    sn = sin_sb[:, s0:s0 + TB]
                    t1 = p1sb.tile([128, TB], F32, tag="t1", name=f"t1_{tb}")
                    t2 = p1sb.tile([128, TB], F32, tag="t2", name=f"t2_{tb}")
                    t3 = p1sb.tile([128, TB], F32, tag="t3", name=f"t3_{tb}")
                    t4 = p1sb.tile([128, TB], F32, tag="t4", name=f"t4_{tb}")
                    nc.vector.tensor_mul(t1[:], psQA[:], cs)
                    nc.vector.tensor_mul(t2[:], psQB[:], sn)
                    nc.vector.tensor_mul(t3[:], psQA[:], sn)
                    nc.vector.tensor_mul(t4[:], psQB[:], cs)
                    Aout = p1sb.tile([128, TB], BF16, tag="Aout", name=f"Ao{tb}")
                    Bout = p1sb.tile([128, TB], BF16, tag="Bout", name=f"Bo{tb}")
                    nc.vector.tensor_sub(Aout[:], t1[:], t2[:])
                    nc.vector.tensor_add(Bout[:], t3[:], t4[:])
                    bb, c0 = divmod(t0, S)
                    for h in range(HPC):
                        rb = (h % 2) * 64
                        nc.vector.tensor_copy(
                            QTb[bb][h // 2][rb:rb + 32, c0:c0 + TB],
                            Aout[32 * h:32 * (h + 1), :])
                        nc.vector.tensor_copy(
                            QTb[bb][h // 2][rb + 32:rb + 64, c0:c0 + TB],
                            Bout[32 * h:32 * (h + 1), :])
                    k1 = p1sb.tile([32, TB], F32, tag="k1", name=f"k1_{tb}")
                    k2 = p1sb.tile([32, TB], F32, tag="k2", name=f"k2_{tb}")
                    k3 = p1sb.tile([32, TB], F32, tag="k3", name=f"k3_{tb}")
                    k4 = p1sb.tile([32, TB], F32, tag="k4", name=f"k4_{tb}")
                    nc.vector.tensor_mul(k1[:], psKV[0:32, :], cs[0:32, :])
                    nc.vector.tensor_mul(k2[:], psKV[32:64, :], sn[0:32, :])
                    nc.vector.tensor_mul(k3[:], psKV[0:32, :], sn[0:32, :])
                    nc.vector.tensor_mul(k4[:], psKV[32:64, :], cs[0:32, :])
                    nc.vector.tensor_sub(KTb[bb][0:32, c0:c0 + TB],
                                         k1[:], k2[:])
                    nc.vector.tensor_add(KTb[bb][32:64, c0:c0 + TB],
                                         k3[:], k4[:])
                    nc.vector.tensor_copy(KTb[bb][64:128, c0:c0 + TB],
                                          KTb[bb][0:64, c0:c0 + TB])

                    vst = p1sb.tile([64, TB], BF16, tag="vst", name=f"vst{tb}")
                    nc.scalar.copy(vst[:], psKV[64:128, :])
                    psV = psATp.tile([128, 4 * 64], BF16, tag="psT",
                                     name=f"psV{tb}")
                    for i in range(4):
                        nc.tensor.transpose(psV[:, 64 * i:64 * (i + 1)],
                                            vst[:, 128 * i:128 * (i + 1)],
                                            ident[0:64, 0:64])
                    for i in range(4):
                        nc.scalar.copy(V_aug[tb * 4 + i][:, 0:64],
                                       psV[:, 64 * i:64 * (i + 1)])

                return [load_dma, load_alloc] + [qkv(dt) for dt in range(DT)] + [rope]

            def p2_head(b, h, psSp, psOp, fillers):
                """Emit attention for (b, h); calls one filler closure after
                each kt/qt iteration to interleave other work."""
                qrows = QTb[b][h // 2][(h % 2) * 64:(h % 2) * 64 + 64, :]
                kbase = (h % 2) * 64
                fi = 0

                def fill():
                    nonlocal fi
                    if fi < len(fillers):
                        fillers[fi]()
                        fi += 1

                expS = []
                for kt in range(KTILES):
                    width = S - 128 * kt
                    e = expp.tile([128, width], BF16, tag=f"expS{kt}",
                                  name=f"expS{kt}")
                    expS.append(e)
                    klhs = KTb[b][kbase:kbase + 64,
                                  128 * kt: 128 * (kt + 1)]
                    for s0 in range(128 * kt, S, QSPAN):
                        w = min(QSPAN, S - s0)
                        ps = psSp.tile([128, QSPAN], F32, tag="psS",
                                       name=f"psS{kt}")
                        for n0 in range(0, w, 512):
                            nw = min(512, w - n0)
                            nc.tensor.matmul(
                                ps[:, n0:n0 + nw], klhs,
                                qrows[:, s0 + n0: s0 + n0 + nw],
                                start=True, stop=True)
                        nc.scalar.activation(
                            e[:, s0 - 128 * kt: s0 - 128 * kt + w],
                            ps[:, 0:w], AF.Exp, scale=0.125)
                    nc.gpsimd.affine_select(
                        out=e[:, 0:128], in_=e[:, 0:128],
                        compare_op=mybir.AluOpType.is_ge, fill=0.0,
                        base=0, pattern=[[1, 128]], channel_multiplier=-1)
                    fill()

                attnT = att.tile([64, S], BF16, tag="attnT", name=f"attnT{b}{h}")
                psO = None
                for qt in range(KTILES):
                    if qt % 4 == 0:
                        psO = psOp.tile([128, 260], F32, tag="psO",
                                        name=f"psO{qt}")
                    c0 = 65 * (qt % 4)
                    for i in range(qt + 1):
                        nc.tensor.matmul(
                            psO[:, c0:c0 + 65],
                            expS[i][:, 128 * (qt - i): 128 * (qt - i) + 128],
                            V_aug[b * KTILES + i][:],
                            start=(i == 0), stop=(i == qt))
                    rc = att.tile([128, 1], F32, tag="rc", name=f"rc{qt}")
                    nc.vector.reciprocal(rc[:], psO[:, c0 + 64:c0 + 65])
                    attn_n = att.tile([128, 64], BF16, tag="attn_n",
                                      name=f"an{qt}")
                    nc.vector.tensor_scalar(attn_n[:], psO[:, c0:c0 + 64], rc[:],
                                            None, mybir.AluOpType.mult)
                    psAT = psATp.tile([64, 128], BF16, tag="psT",
                                      name=f"psAT{qt}")
                    nc.tensor.transpose(psAT[:], attn_n[:], ident[:])
                    nc.vector.tensor_copy(attnT[:, 128 * qt:128 * (qt + 1)],
                                          psAT[:])
                    fill()
                while fi < len(fillers):
                    fill()
                for j in range(NC_CORES):
                    nc.sync.dma_start(
                        a2a_in[b][j, HD * h:HD * (h + 1), :],
                        attnT[:, BSL * j:BSL * (j + 1)])

            def collective(b):
                if timeline:
                    nc.gpsimd.dma_start(a2a_out[b][:], a2a_in[b][:])
                else:
                    nc.gpsimd.collective_compute(
                        "AllToAll", mybir.AluOpType.bypass, replica_groups=rg,
                        ins=[a2a_in[b][:]], outs=[a2a_out[b][:]])

            def p3_batch(b, rcvp, p3sb, psWp, wo_sb):
                rcv = []
                for dt in range(DT):
                    r = rcvp.tile([128, BSL], BF16, tag="rcv",
                                  name=f"rcv{b}_{dt}")
                    nc.sync.dma_start(
                        r[:],
                        a2a_out[b][dt // 2,
                                   (dt % 2) * 128:(dt % 2) * 128 + 128, :])
                    rcv.append(r)
                for tt in range(BSL // 128):
                    psW = [psWp.tile([128, 512], F32, tag="psW",
                                     name=f"psW{b}{tt}{i}") for i in range(4)]
                    for dt in range(DT):
                        for eb in range(4):
                            nc.tensor.matmul(
                                psW[eb][:],
                                rcv[dt][:, 128 * tt:128 * (tt + 1)],
                                wo_sb[dt][:, 512 * eb:512 * (eb + 1)],
                                start=(dt == 0), stop=(dt == DT - 1))
                    for eb in range(4):
                        osb = p3sb.tile([128, 512], F32, tag="osb",
                                        name=f"osb{b}{tt}{eb}")
                        nc.scalar.copy(osb[:], psW[eb][:])
                        nc.sync.dma_start(
                            out[b * BSL + 128 * tt: b * BSL + 128 * (tt + 1),
                                512 * eb:512 * (eb + 1)],
                            osb[:])

            # ---------------- emission ----------------
            with tc.tile_pool(name="p1c", bufs=1) as p1c, \
                 tc.tile_pool(name="p1sb", bufs=1) as p1sb, \
                 tc.tile_pool(name="xbfp", bufs=8) as xbfp, \
                 tc.tile_pool(name="xtp", bufs=4) as xtp, \
                 tc.tile_pool(name="p1ps", bufs=1, space="PSUM") as p1ps, \
                 tc.tile_pool(name="psSa", bufs=2, space="PSUM") as psSa, \
                 tc.tile_pool(name="psOa", bufs=1, space="PSUM") as psOa:
                cos_sb = p1c.tile([128, S], F32, tag="cos")
                sin_sb = p1c.tile([128, S], F32, tag="sin")
                nc.sync.dma_start(cos_sb[:], cos4.ap())
                nc.sync.dma_start(sin_sb[:], sin4.ap())
                wq_sb_A = p1c.tile([128, DT, 128], BF16, tag="wqA")
                wq_sb_B = p1c.tile([128, DT, 128], BF16, tag="wqB")
                wkv_sb = p1c.tile([128, DT, 128], BF16, tag="wkv")
                p1args = (p1sb, xbfp, xtp, p1ps, cos_sb, sin_sb,
                          wq_sb_A, wq_sb_B, wkv_sb)
                chunks0 = p1_chunks(0, *p1args)
                chunks0[0]()          # tb0 x DMAs ahead of weight DMAs
                nc.gpsimd.dma_start(
                    wq_sb_A[:], wqTA.ap().rearrange("(dt p) e -> p dt e", p=128))
                nc.gpsimd.dma_start(
                    wq_sb_B[:], wqTB.ap().rearrange("(dt p) e -> p dt e", p=128))
                nc.gpsimd.dma_start(
                    wkv_sb[:], wkvT.ap().rearrange("(dt p) e -> p dt e", p=128))
                chunks1 = p1_chunks(1, *p1args)
                chunks1[0]()          # tb1 x DMAs prefetch (no psum allocs)
                for c in chunks0[1:]:
                    c()
                for c in chunks1[1:]:
                    c()
                for tb in range(2, 4):
                    for c in p1_chunks(tb, *p1args):
                        c()
                for h in range(HPC):
                    p2_head(0, h, psSa, psOa, p1_chunks(4 + h, *p1args))
            collective(0)

            with tc.tile_pool(name="wo", bufs=DT) as wo_pool:
                wo_sb = []
                for dt in range(DT):
                    w = wo_pool.tile([128, D], BF16, tag="wo", name=f"wo{dt}")
                    nc.gpsimd.dma_start(w[:], woT[128 * dt:128 * (dt + 1), :])
                    wo_sb.append(w)
                with tc.tile_pool(name="psSb", bufs=3, space="PSUM") as psSb, \
                     tc.tile_pool(name="psOb", bufs=2, space="PSUM") as psOb:
                    for h in range(HPC):
                        p2_head(1, h, psSb, psOb, [])
                    collective(1)

                with tc.tile_pool(name="p3sb", bufs=4) as p3sb, \
                     tc.tile_pool(name="rcv", bufs=2 * DT) as rcvp, \
                     tc.tile_pool(name="psW", bufs=4, space="PSUM") as psWp:
                    p3_batch(0, rcvp, p3sb, psWp, wo_sb)
                    p3_batch(1, rcvp, p3sb, psWp, wo_sb)

    nc.compile()
    return nc


def _perm_eo(n):
    return list(range(0, n, 2)) + list(range(1, n, 2))


def host_inputs(x, freqs_cos, freqs_sin, wq, wk, wv, wo):
    x2d = np.ascontiguousarray(np.asarray(x).reshape(T, D), dtype=np.float32)
    fcT = np.asarray(freqs_cos).T.astype(np.float32)
    fsT = np.asarray(freqs_sin).T.astype(np.float32)
    cos4 = np.ascontiguousarray(np.tile(fcT, (4, 1)))
    sin4 = np.ascontiguousarray(np.tile(fsT, (4, 1)))
    woT = np.ascontiguousarray(np.asarray(wo).T, dtype=np.float32)
    wq = np.asarray(wq)
    wk = np.asarray(wk)
    wv = np.asarray(wv)

    permA = [h * HD + 2 * j for h in range(HPC) for j in range(HD // 2)]
    permB = [h * HD + 2 * j + 1 for h in range(HPC) for j in range(HD // 2)]
    permK = _perm_eo(HD)

    in_maps = []
    for c in range(NC_CORES):
        wq_c = wq[EQ * c: EQ * (c + 1)]
        wqTA_ = np.ascontiguousarray(wq_c[permA].T, dtype=np.float32)
        wqTB_ = np.ascontiguousarray(wq_c[permB].T, dtype=np.float32)
        wk_c = wk[HD * c: HD * (c + 1)]
        wv_c = wv[HD * c: HD * (c + 1)]
        wkvT_ = np.ascontiguousarray(
            np.concatenate([wk_c[permK], wv_c], axis=0).T, dtype=np.float32)
        in_maps.append({
            "x": x2d, "cos4": cos4, "sin4": sin4,
            "wqTA": wqTA_, "wqTB": wqTB_, "wkvT": wkvT_, "woT": woT,
        })
    return in_maps


def host_gather(results):
    full = np.zeros((B, S, D), np.float32)
    for c in range(NC_CORES):
        o = results[c]["out"]
        for b in range(B):
            full[b, BSL * c: BSL * (c + 1), :] = o[b * BSL:(b + 1) * BSL]
    return full


_NC_CACHE = None


def _get_nc():
    global _NC_CACHE
    if _NC_CACHE is None:
        _NC_CACHE = build()
    return _NC_CACHE


def kernel(x, freqs_cos, freqs_sin, wq, wk, wv, wo):
    nc = _get_nc()
    in_maps = host_inputs(x, freqs_cos, freqs_sin, wq, wk, wv, wo)
    res = run_bass_kernel_spmd(nc, in_maps, core_ids=list(range(NC_CORES)))
    return host_gather(res.results)



# revision 8
# speedup vs baseline: 1.0268x; 1.0268x over previous
"""Distributed GQA attention kernel for one TRN2 chip (8 NeuronCores).

nn_Attention: B=2, S=2048, D=2048, H=32 q-heads, KV=8 kv-heads, HD=64,
RoPE (interleaved pairs), causal softmax, GQA repeat 4, output proj.

Sharding (tensor-parallel over heads): core c owns q-heads 4c..4c+3 and
kv-head c. x and freq tables replicated (bf16 host-cast: halves HBM).
Per-head attention outputs are exchanged with an AllToAll (bf16) so core c
ends up with the full attention activation for tokens [256c:256c+256) of
each batch, then computes the wo projection for just those tokens.

v2 schedule (vs v1): engines execute in emission order, so the emission is
arranged to keep PE (the binding engine, ~260us of stream time) saturated:
  A. QKV+RoPE for batch-0 tokens, pass-pipelined: for each 512-token block,
     the three projection matmul groups (Q-real, Q-imag, KV) run as
     separate passes over the 16 contraction tiles so the RoPE consumption
     of each PSUM accumulator overlaps the next pass's matmuls (PSUM is
     the scarce resource: 3 banks single-buffered).  x transposed to
     d-major on PE; psum evictions alternate ACT/DVE (phase A) or
     DVE/Pool (as phase-B fillers) so no single engine gates the PE.
  B. Batch-0 attention heads, emission-interleaved with batch-1 QKV passes
     (exp on ACT overlaps projection work on PE/DVE).  Narrow (512-col)
     exp chunks: PSUM budget is 3(Qkv)+1(transp)+2(scores)+2(attn-out).
  C. AllToAll(batch 0); batch-1 attention with WIDE (1024-col) exp chunks
     (phase-1 PSUM freed -> 4 banks of score buffer; fewer ACT fixed
     overheads), interleaved with the batch-0 wo projection (PE filler
     under the ACT-bound exp stretch).  AllToAll(batch 1); wo(batch 1).
"""
from contextlib import ExitStack

import numpy as np

import concourse.bass as bass
import concourse.mybir as mybir
import concourse.tile as tile
from concourse import bacc
from concourse.bass_utils import run_bass_kernel_spmd
from concourse.masks import make_identity

F32 = mybir.dt.float32
BF16 = mybir.dt.bfloat16
AF = mybir.ActivationFunctionType

NC_CORES = 8
B = 2
S = 2048
D = 2048
H = 32
KV = 8
HD = 64
HPC = H // NC_CORES      # 4 q heads per core
EQ = HPC * HD            # 256
T = B * S
TB = 512                 # phase-1 token block
NTB = T // TB
KTILES = S // 128
DT = D // 128
TSLICE = T // NC_CORES
BSL = TSLICE // B        # per-batch token slice each core outputs


def build(reps: int = 1, timeline: bool = False):
    nc = bacc.Bacc("TRN2", target_bir_lowering=False, debug=False,
                   num_devices=NC_CORES)

    x = nc.dram_tensor("x", [T, D], BF16, kind="ExternalInput")
    cos4 = nc.dram_tensor("cos4", [128, S], BF16, kind="ExternalInput")
    sin4 = nc.dram_tensor("sin4", [128, S], BF16, kind="ExternalInput")
    wqTA = nc.dram_tensor("wqTA", [D, 128], BF16, kind="ExternalInput")
    wqTB = nc.dram_tensor("wqTB", [D, 128], BF16, kind="ExternalInput")
    wkvT = nc.dram_tensor("wkvT", [D, 128], BF16, kind="ExternalInput")
    woT = nc.dram_tensor("woT", [D, D], BF16, kind="ExternalInput")
    out = nc.dram_tensor("out", [TSLICE, D], F32, kind="ExternalOutput")

    a2a_in = [nc.dram_tensor(f"a2a_in{b}", [NC_CORES, EQ, BSL], BF16)
              for b in range(B)]
    a2a_out = [nc.dram_tensor(f"a2a_out{b}", [NC_CORES, EQ, BSL], BF16)
               for b in range(B)]
    rg = [list(range(NC_CORES))]

    with tile.TileContext(nc) as tc, ExitStack() as es:
        const = es.enter_context(tc.tile_pool(name="const", bufs=1))
        ident = const.tile([128, 128], BF16, tag="ident")
        make_identity(nc, ident[:])
        # V (token-major, with the softmax-denominator ones column) for all
        # B*KTILES 128-token blocks lives in one tile: 65-col slots, data
        # cols overwritten per rep, ones columns persist from one memset.
        V_all = const.tile([128, B * KTILES * 65], BF16, tag="vall")
        nc.gpsimd.memset(V_all[:], 1.0)

        qt_pool = es.enter_context(tc.tile_pool(name="qt", bufs=1))
        QTb = [[qt_pool.tile([128, S], BF16, tag=f"QT{b}{g}", name=f"QT{b}{g}")
                for g in range(2)] for b in range(B)]
        KTb = [qt_pool.tile([128, S], BF16, tag=f"KT{b}", name=f"KT{b}")
               for b in range(B)]

        for _rep in range(reps):
          with tc.tile_pool(name="att", bufs=2) as att, \
               tc.tile_pool(name="expp", bufs=2) as expp:

            # ---------------- phase-1 chunklets ----------------
            def p1_chunklets(tb, pstp, xbfp, xtp, ropep, psQ, cos_sb, sin_sb,
                             wq_sb_A, wq_sb_B, wkv_sb):
                """Closures emitting phase 1 for token block tb: x DMA, then
                three matmul passes (Q-real / Q-imag / KV) each chased by
                its RoPE consumers so PSUM accumulators recycle quickly."""
                t0 = tb * TB
                bb, c0 = divmod(t0, S)
                st = {}

                def dma():
                    xbf = []
                    for i in range(4):
                        xt_ = xbfp.tile([128, D], BF16, tag="xbf",
                                        name=f"xbf{tb}_{i}")
                        nc.gpsimd.dma_start(
                            xt_[:], x[t0 + 128 * i: t0 + 128 * (i + 1), :])
                        xbf.append(xt_)
                    st["xbf"] = xbf
                    st["xT"] = [None] * DT

                def transpose(dt):
                    psT = pstp.tile([128, TB], BF16, tag="pst",
                                    name=f"psT{tb}_{dt}")
                    for i in range(4):
                        nc.tensor.transpose(
                            psT[:, 128 * i: 128 * (i + 1)],
                            st["xbf"][i][:, 128 * dt: 128 * (dt + 1)],
                            ident[:])
                    xt_ = xtp.tile([128, TB], BF16, tag="xT",
                                   name=f"xT{tb}_{dt}")
                    # spread PSUM->SBUF evictions: phase A (tb<4) ACT/DVE;
                    # phase-B fillers DVE-only (ACT is exp-busy, and the
                    # Pool engine cannot read PSUM)
                    if tb < 4 and dt % 2 == 0:
                        nc.scalar.copy(xt_[:], psT[:])
                    else:
                        nc.vector.tensor_copy(xt_[:], psT[:])
                    st["xT"][dt] = xt_

                def mk_pass(key, w_sb, dts):
                    def f():
                        if key not in st:
                            st[key] = psQ.tile([128, TB], F32, tag=key,
                                               name=f"{key}{tb}")
                        ps = st[key]
                        for dt in dts:
                            if st["xT"][dt] is None:
                                transpose(dt)
                            nc.tensor.matmul(ps[:], w_sb[:, dt, :],
                                             st["xT"][dt][:],
                                             start=(dt == 0),
                                             stop=(dt == DT - 1))
                    return f

                def ropeA():
                    cs = cos_sb[:, c0:c0 + TB]
                    sn = sin_sb[:, c0:c0 + TB]
                    t1 = ropep.tile([128, TB], F32, tag="t1", name=f"t1_{tb}")
                    t3 = ropep.tile([128, TB], F32, tag="t3", name=f"t3_{tb}")
                    nc.vector.tensor_mul(t1[:], st["qa"][:], cs)
                    nc.vector.tensor_mul(t3[:], st["qa"][:], sn)
                    st["t1"], st["t3"] = t1, t3

                def ropeB():
                    cs = cos_sb[:, c0:c0 + TB]
                    sn = sin_sb[:, c0:c0 + TB]
                    t2 = ropep.tile([128, TB], F32, tag="t2", name=f"t2_{tb}")
                    t4 = ropep.tile([128, TB], F32, tag="t4", name=f"t4_{tb}")
                    nc.vector.tensor_mul(t2[:], st["qb"][:], sn)
                    nc.vector.tensor_mul(t4[:], st["qb"][:], cs)
                    Aout = ropep.tile([128, TB], BF16, tag="Ao", name=f"Ao{tb}")
                    Bout = ropep.tile([128, TB], BF16, tag="Bo", name=f"Bo{tb}")
                    nc.vector.tensor_sub(Aout[:], st["t1"][:], t2[:])
                    nc.vector.tensor_add(Bout[:], st["t3"][:], t4[:])
                    for h in range(HPC):
                        rb = (h % 2) * 64
                        nc.vector.tensor_copy(
                            QTb[bb][h // 2][rb:rb + 32, c0:c0 + TB],
                            Aout[32 * h:32 * (h + 1), :])
                        nc.vector.tensor_copy(
                            QTb[bb][h // 2][rb + 32:rb + 64, c0:c0 + TB],
                            Bout[32 * h:32 * (h + 1), :])

                def ropeC():
                    cs = cos_sb[0:32, c0:c0 + TB]
                    sn = sin_sb[0:32, c0:c0 + TB]
                    kv = st["kv"]
                    # column-stacked products (TensorTensor requires equal
                    # base partitions when both operands are in SBUF)
                    kk = ropep.tile([32, 4 * TB], F32, tag="kk", name=f"kk{tb}")
                    nc.vector.tensor_mul(kk[:, 0 * TB:1 * TB], kv[0:32, :], cs)
                    nc.vector.tensor_mul(kk[:, 1 * TB:2 * TB], kv[32:64, :], sn)
                    nc.vector.tensor_mul(kk[:, 2 * TB:3 * TB], kv[0:32, :], sn)
                    nc.vector.tensor_mul(kk[:, 3 * TB:4 * TB], kv[32:64, :], cs)
                    nc.vector.tensor_sub(KTb[bb][0:32, c0:c0 + TB],
                                         kk[:, 0 * TB:1 * TB],
                                         kk[:, 1 * TB:2 * TB])
                    nc.vector.tensor_add(KTb[bb][32:64, c0:c0 + TB],
                                         kk[:, 2 * TB:3 * TB],
                                         kk[:, 3 * TB:4 * TB])
                    # odd heads read K (and their Q lives) at partitions
                    # 64:128 -- contract partitions must line up
                    nc.vector.tensor_copy(KTb[bb][64:128, c0:c0 + TB],
                                          KTb[bb][0:64, c0:c0 + TB])
                    vst = ropep.tile([64, TB], BF16, tag="vst",
                                     name=f"vst{tb}")
                    nc.scalar.copy(vst[:], kv[64:128, :])
                    psV = pstp.tile([128, 4 * 64], BF16, tag="pst",
                                    name=f"psV{tb}")
                    for i in range(4):
                        nc.tensor.transpose(psV[:, 64 * i:64 * (i + 1)],
                                            vst[:, 128 * i:128 * (i + 1)],
                                            ident[0:64, 0:64])
                    slot0 = bb * KTILES + (t0 % S) // 128
                    dst = V_all.rearrange("p (k c) -> p k c", c=65)
                    nc.scalar.copy(
                        dst[:, slot0:slot0 + 4, 0:64],
                        psV[:].rearrange("p (k c) -> p k c", c=64))

                return [dma,
                        mk_pass("qa", wq_sb_A, range(0, 8)),
                        mk_pass("qa", wq_sb_A, range(8, DT)),
                        ropeA,
                        mk_pass("qb", wq_sb_B, range(0, 8)),
                        mk_pass("qb", wq_sb_B, range(8, DT)),
                        ropeB,
                        mk_pass("kv", wkv_sb, range(0, 8)),
                        mk_pass("kv", wkv_sb, range(8, DT)),
                        ropeC]

            # ---------------- attention ----------------
            def p2_head(b, h, psSp, psOp, psTp, fillers, span):
                qrows = QTb[b][h // 2][(h % 2) * 64:(h % 2) * 64 + 64, :]
                kbase = (h % 2) * 64
                fi = 0

                def fill():
                    nonlocal fi
                    if fi < len(fillers):
                        fillers[fi]()
                        fi += 1

                expS = []
                for kt in range(KTILES):
                    width = S - 128 * kt
                    e = expp.tile([128, width], BF16, tag=f"expS{kt}",
                                  name=f"expS{kt}")
                    expS.append(e)
                    klhs = KTb[b][kbase:kbase + 64,
                                  128 * kt: 128 * (kt + 1)]
                    for s0 in range(128 * kt, S, span):
                        w = min(span, S - s0)
                        ps = psSp.tile([128, span], F32, tag="psS",
                                       name=f"psS{kt}")
                        for n0 in range(0, w, 512):
                            nw = min(512, w - n0)
                            nc.tensor.matmul(
                                ps[:, n0:n0 + nw], klhs,
                                qrows[:, s0 + n0: s0 + n0 + nw],
                                start=True, stop=True)
                        nc.scalar.activation(
                            e[:, s0 - 128 * kt: s0 - 128 * kt + w],
                            ps[:, 0:w], AF.Exp, scale=0.125)
                    nc.gpsimd.affine_select(
                        out=e[:, 0:128], in_=e[:, 0:128],
                        compare_op=mybir.AluOpType.is_ge, fill=0.0,
                        base=0, pattern=[[1, 128]], channel_multiplier=-1)
                    fill()

                attnT = att.tile([64, S], BF16, tag="attnT", name=f"attnT{b}{h}")
                psO = None
                for qt in range(KTILES):
                    if qt % 4 == 0:
                        psO = psOp.tile([128, 260], F32, tag="psO",
                                        name=f"psO{qt}",
                                        padded_shape=[128, 512])
                    c0 = 65 * (qt % 4)
                    for i in range(qt + 1):
                        v0 = 65 * (b * KTILES + i)
                        nc.tensor.matmul(
                            psO[:, c0:c0 + 65],
                            expS[i][:, 128 * (qt - i): 128 * (qt - i) + 128],
                            V_all[:, v0:v0 + 65],
                            start=(i == 0), stop=(i == qt))
                    rc = att.tile([128, 1], F32, tag="rc", name=f"rc{qt}")
                    nc.vector.reciprocal(rc[:], psO[:, c0 + 64:c0 + 65])
                    attn_n = att.tile([128, 64], BF16, tag="attn_n",
                                      name=f"an{qt}")
                    nc.vector.tensor_scalar(attn_n[:], psO[:, c0:c0 + 64], rc[:],
                                            None, mybir.AluOpType.mult)
                    psAT = psTp.tile([64, 128], BF16, tag="pst",
                                     name=f"psAT{qt}")
                    nc.tensor.transpose(psAT[:], attn_n[:], ident[:])
                    nc.vector.tensor_copy(attnT[:, 128 * qt:128 * (qt + 1)],
                                          psAT[:])
                    fill()
                while fi < len(fillers):
                    fill()
                for j in range(NC_CORES):
                    nc.sync.dma_start(
                        a2a_in[b][j, HD * h:HD * (h + 1), :],
                        attnT[:, BSL * j:BSL * (j + 1)])

            def collective(b):
                if timeline:
                    nc.gpsimd.dma_start(a2a_out[b][:], a2a_in[b][:])
                else:
                    nc.gpsimd.collective_compute(
                        "AllToAll", mybir.AluOpType.bypass, replica_groups=rg,
                        ins=[a2a_in[b][:]], outs=[a2a_out[b][:]])

            # ---------------- output projection ----------------
            def p3_chunklets(b, rcvp, p3sb, psWp, wo_sb):
                rcv = []

                def dmas():
                    for dt in range(DT):
                        r = rcvp.tile([128, BSL], BF16, tag="rcv",
                                      name=f"rcv{b}_{dt}")
                        nc.sync.dma_start(
                            r[:],
                            a2a_out[b][dt // 2,
                                       (dt % 2) * 128:(dt % 2) * 128 + 128, :])
                        rcv.append(r)

                def tt_eb(tt, eb):
                    def f():
                        psW = psWp.tile([128, 512], F32, tag="psW",
                                        name=f"psW{b}{tt}{eb}")
                        for dt in range(DT):
                            nc.tensor.matmul(
                                psW[:],
                                rcv[dt][:, 128 * tt:128 * (tt + 1)],
                                wo_sb[dt][:, 512 * eb:512 * (eb + 1)],
                                start=(dt == 0), stop=(dt == DT - 1))
                        osb = p3sb.tile([128, 512], F32, tag="osb",
                                        name=f"osb{b}{tt}{eb}")
                        nc.scalar.copy(osb[:], psW[:])
                        nc.sync.dma_start(
                            out[b * BSL + 128 * tt: b * BSL + 128 * (tt + 1),
                                512 * eb:512 * (eb + 1)],
                            osb[:])
                    return f

                return [dmas] + [tt_eb(tt, eb)
                                 for tt in range(BSL // 128)
                                 for eb in range(4)]

            # ---------------- emission ----------------
            with tc.tile_pool(name="p1c", bufs=1) as p1c, \
                 tc.tile_pool(name="xbfp", bufs=8) as xbfp, \
                 tc.tile_pool(name="xtp", bufs=17) as xtp, \
                 tc.tile_pool(name="ropep", bufs=1) as ropep, \
                 tc.tile_pool(name="pst", bufs=2, space="PSUM") as pstp, \
                 tc.tile_pool(name="psQ", bufs=1, space="PSUM") as psQ, \
                 tc.tile_pool(name="psSa", bufs=2, space="PSUM") as psSa, \
                 tc.tile_pool(name="psOa", bufs=1, space="PSUM") as psOa:
                cos_sb = p1c.tile([128, S], BF16, tag="cos")
                sin_sb = p1c.tile([128, S], BF16, tag="sin")
                nc.sync.dma_start(cos_sb[:], cos4.ap())
                nc.sync.dma_start(sin_sb[:], sin4.ap())
                wq_sb_A = p1c.tile([128, DT, 128], BF16, tag="wqA")
                wq_sb_B = p1c.tile([128, DT, 128], BF16, tag="wqB")
                wkv_sb = p1c.tile([128, DT, 128], BF16, tag="wkv")
                p1args = (pstp, xbfp, xtp, ropep, psQ, cos_sb, sin_sb,
                          wq_sb_A, wq_sb_B, wkv_sb)
                chunks0 = p1_chunklets(0, *p1args)
                chunks0[0]()          # tb0 x DMAs ahead of weight DMAs
                nc.gpsimd.dma_start(
                    wq_sb_A[:], wqTA.ap().rearrange("(dt p) e -> p dt e", p=128))
                nc.gpsimd.dma_start(
                    wq_sb_B[:], wqTB.ap().rearrange("(dt p) e -> p dt e", p=128))
                nc.gpsimd.dma_start(
                    wkv_sb[:], wkvT.ap().rearrange("(dt p) e -> p dt e", p=128))
                chunks1 = p1_chunklets(1, *p1args)
                chunks1[0]()          # tb1 x DMAs prefetch
                for c in chunks0[1:]:
                    c()
                for c in chunks1[1:]:
                    c()
                for tb in range(2, 4):
                    for c in p1_chunklets(tb, *p1args):
                        c()
                for h in range(HPC):
                    p2_head(0, h, psSa, psOa, pstp,
                            p1_chunklets(4 + h, *p1args), span=512)
            collective(0)

            with tc.tile_pool(name="wo", bufs=DT) as wo_pool, \
                 tc.tile_pool(name="rcv", bufs=2 * DT) as rcvp, \
                 tc.tile_pool(name="p3sb", bufs=4) as p3sb:
                wo_sb = []
                for dt in range(DT):
                    w = wo_pool.tile([128, D], BF16, tag="wo", name=f"wo{dt}")
                    nc.gpsimd.dma_start(w[:], woT[128 * dt:128 * (dt + 1), :])
                    wo_sb.append(w)
                with tc.tile_pool(name="psSb", bufs=2, space="PSUM") as psSb, \
                     tc.tile_pool(name="psOb", bufs=1, space="PSUM") as psOb, \
                     tc.tile_pool(name="psATc", bufs=1, space="PSUM") as psATc, \
                     tc.tile_pool(name="psW", bufs=2, space="PSUM") as psWp:
                    f3 = p3_chunklets(0, rcvp, p3sb, psWp, wo_sb)
                    for h in range(HPC):
                        # give the collective a head start before wo(0)
                        # fillers begin pulling on a2a_out(0)
                        fill_h = ([] if h == 0 else
                                  f3[3 * (h - 1):3 * h] if h < 3 else
                                  f3[6:])
                        p2_head(1, h, psSb, psOb, psATc, fill_h,
                                span=1024)
                    collective(1)
                    for c in p3_chunklets(1, rcvp, p3sb, psWp, wo_sb):
                        c()

    nc.compile()
    return nc


def _perm_eo(n):
    return list(range(0, n, 2)) + list(range(1, n, 2))


def _bf16(a):
    import ml_dtypes
    return np.ascontiguousarray(np.asarray(a, dtype=np.float32)).astype(
        ml_dtypes.bfloat16)


def host_inputs(x, freqs_cos, freqs_sin, wq, wk, wv, wo):
    x2d = _bf16(np.asarray(x).reshape(T, D))
    fcT = np.asarray(freqs_cos).T.astype(np.float32)
    fsT = np.asarray(freqs_sin).T.astype(np.float32)
    cos4 = _bf16(np.tile(fcT, (4, 1)))
    sin4 = _bf16(np.tile(fsT, (4, 1)))
    woT = _bf16(np.asarray(wo).T)
    wq = np.asarray(wq)
    wk = np.asarray(wk)
    wv = np.asarray(wv)

    permA = [h * HD + 2 * j for h in range(HPC) for j in range(HD // 2)]
    permB = [h * HD + 2 * j + 1 for h in range(HPC) for j in range(HD // 2)]
    permK = _perm_eo(HD)

    in_maps = []
    for c in range(NC_CORES):
        wq_c = wq[EQ * c: EQ * (c + 1)]
        wqTA_ = _bf16(wq_c[permA].T)
        wqTB_ = _bf16(wq_c[permB].T)
        wk_c = wk[HD * c: HD * (c + 1)]
        wv_c = wv[HD * c: HD * (c + 1)]
        wkvT_ = _bf16(np.concatenate([wk_c[permK], wv_c], axis=0).T)
        in_maps.append({
            "x": x2d, "cos4": cos4, "sin4": sin4,
            "wqTA": wqTA_, "wqTB": wqTB_, "wkvT": wkvT_, "woT": woT,
        })
    return in_maps


def host_gather(results):
    full = np.zeros((B, S, D), np.float32)
    for c in range(NC_CORES):
        o = results[c]["out"]
        for b in range(B):
            full[b, BSL * c: BSL * (c + 1), :] = o[b * BSL:(b + 1) * BSL]
    return full


_NC_CACHE = None


def _get_nc():
    global _NC_CACHE
    if _NC_CACHE is None:
        _NC_CACHE = build()
    return _NC_CACHE


def kernel(x, freqs_cos, freqs_sin, wq, wk, wv, wo):
    nc = _get_nc()
    in_maps = host_inputs(x, freqs_cos, freqs_sin, wq, wk, wv, wo)
    res = run_bass_kernel_spmd(nc, in_maps, core_ids=list(range(NC_CORES)))
    return host_gather(res.results)
